# revision 13
# baseline (speedup 1.0000x reference)
"""FAVOR+ attention kernel for 8 Trainium2 NeuronCores.

Sharding: core c handles batch b = c//2 and head-group g = c%2 (8 of 16
heads). Each core computes its group's partial output projection
out_part^T [D, N]; the host sums the two partials per batch and
transposes back.

Layout convention on-chip: "feature-major" (transposed) tensors — the
qkv projections produce qT/kT [j, n] with j on partitions, which feeds
the Performer feature map and the final projection without any on-chip
transposes (x arrives pre-transposed from the host).
"""

import math

import numpy as np

B, N, D = 4, 4096, 1024
H = 16
HD = 64
M = HD
HPC = 8          # heads per core
G = HPC * HD     # 512 cols per group per q/k/v
STAB = 1e-6
RATIO = 1.0 / math.sqrt(M)
EXP_BIAS = STAB + math.log(RATIO)   # exp(x + STAB) * ratio == exp(x + STAB + ln ratio)

CHUNK = 512
NT = 128




def _build(n_tokens=N):
    import concourse.bass as bass
    import concourse.mybir as mybir
    import concourse.tile as tile
    from concourse import bacc

    f32 = mybir.dt.float32
    f32r = mybir.dt.float32r
    AF = mybir.ActivationFunctionType

    nchunks = n_tokens // CHUNK
    ntiles = CHUNK // NT  # 4

    nc = bacc.Bacc("TRN2", target_bir_lowering=False, debug=False, num_devices=8)

    xT = nc.dram_tensor("xT", [D, n_tokens], f32r, kind="ExternalInput")
    w_qkv = nc.dram_tensor("w_qkv", [D, 3 * G], f32r, kind="ExternalInput")
    b_qk = nc.dram_tensor("b_qk", [HD, 2 * HPC], f32, kind="ExternalInput")
    w_proj = nc.dram_tensor("w_proj", [G, D], f32r, kind="ExternalInput")
    p2 = nc.dram_tensor("p2", [HD, M], f32r, kind="ExternalInput")
    nh2 = nc.dram_tensor("nh2", [HD, M], f32r, kind="ExternalInput")
    bvb = nc.dram_tensor("bvb", [128, HPC * 66], f32, kind="ExternalInput")
    pout = nc.dram_tensor("pout", [D, n_tokens], f32, kind="ExternalOutput")

    with tile.TileContext(nc) as tc:
        with (
            tc.tile_pool(name="const", bufs=1) as cpool,
            tc.tile_pool(name="kv_sb", bufs=1) as kvsb_pool,
        ):
            # weights resident in SBUF
            w_sb = cpool.tile([128, 8, 3 * G], f32r)
            nc.sync.dma_start(w_sb[:], w_qkv.ap().rearrange("(dt p) j -> p dt j", p=128))
            wp_sb = cpool.tile([128, 4, D], f32r)
            nc.sync.dma_start(wp_sb[:], w_proj.ap().rearrange("(ht p) o -> p ht o", p=128))
            # NOTE every fp32r matmul operand must sit at partition base 0:
            # mixing lhsT base partitions across fp32r matmuls hangs the
            # device (probed on HW). Hence the per-head [64, ...] layouts.
            p_sb = cpool.tile([HD, M], f32r)
            nc.sync.dma_start(p_sb[:], p2[:])
            bqk_sb = cpool.tile([HD, 2 * HPC], f32)
            nc.sync.dma_start(bqk_sb[:], b_qk[:])
            bvb_sb = cpool.tile([128, HPC, 66], f32)
            nc.sync.dma_start(bvb_sb[:], bvb.ap().rearrange("p (h e) -> p h e", h=HPC))
            nh_sb = cpool.tile([HD, M], f32r)
            nc.sync.dma_start(nh_sb[:], nh2[:])
            ebias_sb = cpool.tile([128, 1], f32)
            nc.vector.memset(ebias_sb[:], EXP_BIAS)

            kvT_sb = kvsb_pool.tile([HD, HPC, 66], f32r)

            # ---------------- PASS 1: k', v -> kv state ----------------
            with (
                tc.tile_pool(name="xt1", bufs=2) as xt_pool,
                tc.tile_pool(name="ksb", bufs=2) as k_pool,
                tc.tile_pool(name="ksq", bufs=1) as ksq_pool,
                tc.tile_pool(name="vext", bufs=2) as v_pool,
                tc.tile_pool(name="kprime", bufs=2) as kp_pool,
                tc.tile_pool(name="ps_kq1", bufs=2, space="PSUM") as ps_kq,
                tc.tile_pool(name="ps_v", bufs=2, space="PSUM") as ps_v,
                tc.tile_pool(name="ps_kproj", bufs=2, space="PSUM") as ps_kproj,
                tc.tile_pool(name="ps_kv", bufs=1, space="PSUM") as ps_kv,
            ):
                kv_ps_a = ps_kv.tile([HD, 4, 66], f32, tag="kva")
                kv_ps_b = ps_kv.tile([HD, 4, 66], f32, tag="kvb")
                kv_ps = (kv_ps_a, kv_ps_b)

                for c in range(nchunks):
                    xt = xt_pool.tile([128, 8, CHUNK], f32r)
                    nc.sync.dma_start(
                        xt[:],
                        xT.ap().rearrange("(dt p) n -> p dt n", p=128)[
                            :, :, c * CHUNK : (c + 1) * CHUNK
                        ],
                    )
                    k_sb = k_pool.tile([64, 8, CHUNK], f32r)
                    ksq_sb = ksq_pool.tile([64, 8, CHUNK], f32r)
                    for jt in range(4):
                        ps = ps_kq.tile([128, CHUNK], f32)
                        for dt in range(8):
                            nc.tensor.matmul(
                                ps[:],
                                (w_sb[:, dt, G + jt * 128 : G + (jt + 1) * 128]),
                                (xt[:, dt, :]),
                                start=(dt == 0),
                                stop=(dt == 7),
                            )
                        for hh in range(2):
                            h = 2 * jt + hh
                            nc.scalar.activation(
                                k_sb[:, h, :], ps[hh * 64 : hh * 64 + 64, :],
                                AF.Identity, bias=bqk_sb[:, HPC + h : HPC + h + 1],
                            )
                            nc.scalar.activation(
                                ksq_sb[:, h, :], k_sb[:, h, :], AF.Square,
                            )

                    for nt in range(ntiles):
                        nsl = slice(nt * NT, (nt + 1) * NT)
                        vps = ps_v.tile([128, G], f32)
                        for dt in range(8):
                            nc.tensor.matmul(
                                vps[:],
                                (xt[:, dt, nsl]),
                                (w_sb[:, dt, 2 * G : 3 * G]),
                                start=(dt == 0),
                                stop=(dt == 7),
                            )
                        vext = v_pool.tile([128, HPC, 66], f32r)
                        nc.vector.tensor_add(
                            vext[:, :, 0:64],
                            vps.rearrange("p (h d) -> p h d", h=HPC),
                            bvb_sb[:, :, 0:64],
                        )
                        nc.vector.tensor_copy(vext[:, :, 64:66], bvb_sb[:, :, 64:66])

                        kpps = ps_kproj.tile([128, G], f32)
                        for h in range(HPC):
                            osl = kpps[:, h * 64 : (h + 1) * 64]
                            nc.tensor.matmul(
                                osl, (k_sb[:, h, nsl]), (p_sb[:]),
                                start=True, stop=False,
                            )
                            nc.tensor.matmul(
                                osl, (ksq_sb[:, h, nsl]), (nh_sb[:]),
                                start=False, stop=True,
                            )
                        kp = kp_pool.tile([128, G], f32r)
                        nc.scalar.activation(kp[:], kpps[:], AF.Exp, bias=ebias_sb[:])

                        first = c == 0 and nt == 0
                        last = c == nchunks - 1 and nt == ntiles - 1
                        for h in range(HPC):
                            # one accumulation group per psum bank: start only
                            # with the bank's first matmul, stop with its last
                            nc.tensor.matmul(
                                kv_ps[h // 4][:, h % 4, :],
                                (kp[:, h * 64 : (h + 1) * 64]),
                                (vext[:, h, :]),
                                start=first and h % 4 == 0,
                                stop=last and h % 4 == 3,
                            )

                nc.vector.tensor_copy(kvT_sb[:, 0:4, :], kv_ps_a[:])
                nc.vector.tensor_copy(kvT_sb[:, 4:8, :], kv_ps_b[:])

            # ---------------- PASS 2: q' -> out -> projection ----------------
            with (
                tc.tile_pool(name="xt2", bufs=2) as xt_pool,
                tc.tile_pool(name="qsb", bufs=2) as q_pool,
                tc.tile_pool(name="qsq", bufs=1) as qsq_pool,
                tc.tile_pool(name="qprime", bufs=1) as qp_pool,
                tc.tile_pool(name="normrow", bufs=2) as nr_pool,
                tc.tile_pool(name="recipb", bufs=2) as rb_pool,
                tc.tile_pool(name="ohT", bufs=2) as oh_pool,
                tc.tile_pool(name="ps_big2", bufs=3, space="PSUM") as ps_big,
                tc.tile_pool(name="ps_s64", bufs=3, space="PSUM") as ps_s64,
            ):
                for c in range(nchunks):
                    xt = xt_pool.tile([128, 8, CHUNK], f32r)
                    nc.sync.dma_start(
                        xt[:],
                        xT.ap().rearrange("(dt p) n -> p dt n", p=128)[
                            :, :, c * CHUNK : (c + 1) * CHUNK
                        ],
                    )
                    q_sb = q_pool.tile([64, 8, CHUNK], f32r)
                    qsq_sb = qsq_pool.tile([64, 8, CHUNK], f32r)
                    for jt in range(4):
                        ps = ps_big.tile([128, CHUNK], f32, tag="big")
                        for dt in range(8):
                            nc.tensor.matmul(
                                ps[:],
                                (w_sb[:, dt, jt * 128 : (jt + 1) * 128]),
                                (xt[:, dt, :]),
                                start=(dt == 0),
                                stop=(dt == 7),
                            )
                        for hh in range(2):
                            h = 2 * jt + hh
                            nc.scalar.activation(
                                q_sb[:, h, :], ps[hh * 64 : hh * 64 + 64, :],
                                AF.Identity, bias=bqk_sb[:, h : h + 1],
                            )
                            nc.scalar.activation(
                                qsq_sb[:, h, :], q_sb[:, h, :], AF.Square,
                            )

                    qprime = qp_pool.tile([64, HPC, CHUNK], f32r)
                    for h in range(HPC):
                        qps = ps_s64.tile([64, CHUNK], f32, tag="s64")
                        nc.tensor.matmul(
                            qps[:], (p_sb[:]), (q_sb[:, h, :]),
                            start=True, stop=False,
                        )
                        nc.tensor.matmul(
                            qps[:], (nh_sb[:]), (qsq_sb[:, h, :]),
                            start=False, stop=True,
                        )
                        nc.scalar.activation(qprime[:, h, :], qps[:], AF.Exp, bias=ebias_sb[0:64, :])

                    # out^T rows 0:64 = unnormalized head output, row 64 =
                    # the normalizer (from the ksum column folded into kv)
                    ohT = oh_pool.tile([128, 4, CHUNK], f32r)
                    for h in range(HPC):
                        ot = ps_s64.tile([65, CHUNK], f32, tag="s64")
                        nc.tensor.matmul(
                            ot[:], (kvT_sb[:, h, 0:65]),
                            (qprime[:, h, :]),
                        )
                        nsc = nr_pool.tile([1, CHUNK], f32, tag="nsc")
                        nc.vector.tensor_scalar_add(nsc[:], ot[64:65, :], 1e-6)
                        rsc = nr_pool.tile([1, CHUNK], f32, tag="rsc")
                        nc.vector.reciprocal_approx_fast(rsc[:], nsc[:])
                        rb = rb_pool.tile([64, CHUNK], f32)
                        nc.gpsimd.partition_broadcast(rb[:], rsc[:])
                        nc.vector.tensor_mul(
                            ohT[(h % 2) * 64 : (h % 2) * 64 + 64, h // 2, :],
                            ot[0:64, :], rb[:],
                        )

                    for otile in range(8):
                        fps = ps_big.tile([128, CHUNK], f32, tag="big")
                        for ht in range(4):
                            nc.tensor.matmul(
                                fps[:],
                                (wp_sb[:, ht, otile * 128 : (otile + 1) * 128]),
                                (ohT[:, ht, :]),
                                start=(ht == 0),
                                stop=(ht == 3),
                            )
                        fout = oh_pool.tile([128, CHUNK], f32, tag="fout")
                        nc.any.tensor_copy(fout[:], fps[:])
                        nc.sync.dma_start(
                            pout[otile * 128 : (otile + 1) * 128,
                                 c * CHUNK : (c + 1) * CHUNK],
                            fout[:],
                        )

    nc.compile()
    return nc


_NC_CACHE = {}


def _get_nc(n_tokens=N):
    if n_tokens not in _NC_CACHE:
        _NC_CACHE[n_tokens] = _build(n_tokens)
    return _NC_CACHE[n_tokens]


def _make_in_maps(x, w_qkv, b_qkv, w_proj, proj_mat):
    """Host-side sharding: one input map per core."""
    in_maps = []
    for c in range(8):
        b, g = c // 2, c % 2
        gs = slice(g * G, (g + 1) * G)
        wq = w_qkv[:, 0:D][:, gs]
        wk = w_qkv[:, D : 2 * D][:, gs]
        wv = w_qkv[:, 2 * D : 3 * D][:, gs]
        bq = b_qkv[0:D][gs]
        bk = b_qkv[D : 2 * D][gs]
        bv = b_qkv[2 * D : 3 * D][gs]
        bvb = np.zeros((128, HPC, 66), np.float32)
        for h in range(HPC):
            bvb[:, h, 0:64] = bv[h * 64 : (h + 1) * 64][None, :]
            bvb[:, h, 64] = 1.0
        bqk = np.concatenate(
            [bq.reshape(HPC, HD).T, bk.reshape(HPC, HD).T], axis=1
        )  # [64, 16]: cols 0..7 q-head biases, 8..15 k-head biases
        in_maps.append(
            {
                "xT": np.ascontiguousarray(x[b].T),
                "w_qkv": np.ascontiguousarray(np.concatenate([wq, wk, wv], axis=1)),
                "b_qk": np.ascontiguousarray(bqk.astype(np.float32)),
                "w_proj": np.ascontiguousarray(w_proj[gs, :]),
                "p2": np.ascontiguousarray(proj_mat),
                "nh2": np.full((HD, M), -0.5, np.float32),
                "bvb": bvb.reshape(128, HPC * 66),
            }
        )
    return in_maps


def kernel(x, w_qkv, b_qkv, w_proj, b_proj, proj_mat, trace=False):
    from concourse.bass_utils import run_bass_kernel_spmd

    x = np.asarray(x, np.float32)
    w_qkv = np.asarray(w_qkv, np.float32)
    b_qkv = np.asarray(b_qkv, np.float32)
    w_proj = np.asarray(w_proj, np.float32)
    b_proj = np.asarray(b_proj, np.float32)
    proj_mat = np.asarray(proj_mat, np.float32)

    nc = _get_nc(N)
    in_maps = _make_in_maps(x, w_qkv, b_qkv, w_proj, proj_mat)
    res = run_bass_kernel_spmd(nc, in_maps, core_ids=list(range(8)), trace=trace)
    out = np.empty((B, N, D), np.float32)
    for b in range(B):
        pT = res.results[2 * b]["pout"] + res.results[2 * b + 1]["pout"]
        out[b] = pT.T + b_proj[None, :]
    kernel.last_result = res
    return out


# revision 14
# speedup vs baseline: 1.1400x; 1.1400x over previous
"""FAVOR+ attention kernel for 8 Trainium2 NeuronCores.

Sharding: core c handles batch b = c//2 and head-group g = c%2 (8 of 16
heads). Each core computes its group's partial output projection
out_part^T [D, N]; the host sums the two partials per batch and
transposes back.

Layout convention on-chip: "feature-major" (transposed) tensors — the
qkv projections produce qT/kT [j, n] with j on partitions, which feeds
the Performer feature map and the final projection without any on-chip
transposes (x arrives pre-transposed from the host).
"""

import math

import numpy as np

B, N, D = 4, 4096, 1024
H = 16
HD = 64
M = HD
HPC = 8          # heads per core
G = HPC * HD     # 512 cols per group per q/k/v
STAB = 1e-6
RATIO = 1.0 / math.sqrt(M)
EXP_BIAS = STAB + math.log(RATIO)   # exp(x + STAB) * ratio == exp(x + STAB + ln ratio)

CHUNK = 512
NT = 128




def _build(n_tokens=N):
    import concourse.bass as bass
    import concourse.mybir as mybir
    import concourse.tile as tile
    from concourse import bacc

    f32 = mybir.dt.float32
    f32r = mybir.dt.float32r
    AF = mybir.ActivationFunctionType

    nchunks = n_tokens // CHUNK
    ntiles = CHUNK // NT  # 4

    nc = bacc.Bacc("TRN2", target_bir_lowering=False, debug=False, num_devices=8)

    xT = nc.dram_tensor("xT", [D, n_tokens], f32r, kind="ExternalInput")
    w_qkv = nc.dram_tensor("w_qkv", [D, 3 * G], f32r, kind="ExternalInput")
    b_qk = nc.dram_tensor("b_qk", [HD, 2 * HPC], f32, kind="ExternalInput")
    w_proj = nc.dram_tensor("w_proj", [G, D], f32r, kind="ExternalInput")
    extp = nc.dram_tensor("extp", [128, M], f32r, kind="ExternalInput")
    bvb = nc.dram_tensor("bvb", [128, HPC * 66], f32, kind="ExternalInput")
    pout = nc.dram_tensor("pout", [D, n_tokens], f32, kind="ExternalOutput")

    with tile.TileContext(nc) as tc:
        with (
            tc.tile_pool(name="const", bufs=1) as cpool,
            tc.tile_pool(name="kv_sb", bufs=1) as kvsb_pool,
        ):
            # weights resident in SBUF
            w_sb = cpool.tile([128, 8, 3 * G], f32r)
            nc.sync.dma_start(w_sb[:], w_qkv.ap().rearrange("(dt p) j -> p dt j", p=128))
            wp_sb = cpool.tile([128, 4, D], f32r)
            nc.sync.dma_start(wp_sb[:], w_proj.ap().rearrange("(ht p) o -> p ht o", p=128))
            # NOTE every fp32r matmul operand must sit at partition base 0:
            # mixing lhsT base partitions across fp32r matmuls hangs the
            # device (probed on HW). extP = [P ; -0.5] so that
            # extP.T @ [k ; k^2] = P.T k - 0.5*sum(k^2) in one matmul with
            # lhsT/rhs both at base 0.
            extp_sb = cpool.tile([128, M], f32r)
            nc.sync.dma_start(extp_sb[:], extp[:])
            bqk_sb = cpool.tile([HD, 2 * HPC], f32)
            nc.sync.dma_start(bqk_sb[:], b_qk[:])
            bvb_sb = cpool.tile([128, HPC, 66], f32)
            nc.sync.dma_start(bvb_sb[:], bvb.ap().rearrange("p (h e) -> p h e", h=HPC))
            ebias_sb = cpool.tile([128, 1], f32)
            nc.vector.memset(ebias_sb[:], EXP_BIAS)

            kvT_sb = kvsb_pool.tile([HD, HPC, 66], f32r)

            # ---------------- PASS 1: k', v -> kv state ----------------
            with (
                tc.tile_pool(name="xt1", bufs=2) as xt_pool,
                tc.tile_pool(name="ksb", bufs=2) as k_pool,
                tc.tile_pool(name="vext", bufs=2) as v_pool,
                tc.tile_pool(name="kprime", bufs=2) as kp_pool,
                tc.tile_pool(name="ps_kq1", bufs=2, space="PSUM") as ps_kq,
                tc.tile_pool(name="ps_v", bufs=2, space="PSUM") as ps_v,
                tc.tile_pool(name="ps_kproj", bufs=2, space="PSUM") as ps_kproj,
                tc.tile_pool(name="ps_kv", bufs=1, space="PSUM") as ps_kv,
            ):
                kv_ps_a = ps_kv.tile([HD, 4, 66], f32, tag="kva")
                kv_ps_b = ps_kv.tile([HD, 4, 66], f32, tag="kvb")
                kv_ps = (kv_ps_a, kv_ps_b)

                for c in range(nchunks):
                    xt = xt_pool.tile([128, 8, CHUNK], f32r)
                    nc.sync.dma_start(
                        xt[:],
                        xT.ap().rearrange("(dt p) n -> p dt n", p=128)[
                            :, :, c * CHUNK : (c + 1) * CHUNK
                        ],
                    )
                    k_sb = k_pool.tile([128, 8, CHUNK], f32r)
                    for jt in range(4):
                        ps = ps_kq.tile([128, CHUNK], f32)
                        for dt in range(8):
                            nc.tensor.matmul(
                                ps[:],
                                (w_sb[:, dt, G + jt * 128 : G + (jt + 1) * 128]),
                                (xt[:, dt, :]),
                                start=(dt == 0),
                                stop=(dt == 7),
                            )
                        for hh in range(2):
                            h = 2 * jt + hh
                            nc.scalar.activation(
                                k_sb[0:64, h, :], ps[hh * 64 : hh * 64 + 64, :],
                                AF.Identity, bias=bqk_sb[:, HPC + h : HPC + h + 1],
                            )
                            nc.scalar.activation(
                                k_sb[64:128, h, :], k_sb[0:64, h, :], AF.Square,
                            )

                    for nt in range(ntiles):
                        nsl = slice(nt * NT, (nt + 1) * NT)
                        vps = ps_v.tile([128, G], f32)
                        for dt in range(8):
                            nc.tensor.matmul(
                                vps[:],
                                (xt[:, dt, nsl]),
                                (w_sb[:, dt, 2 * G : 3 * G]),
                                start=(dt == 0),
                                stop=(dt == 7),
                            )
                        vext = v_pool.tile([128, HPC, 66], f32r)
                        nc.vector.tensor_add(
                            vext[:, :, 0:64],
                            vps.rearrange("p (h d) -> p h d", h=HPC),
                            bvb_sb[:, :, 0:64],
                        )
                        nc.vector.tensor_copy(vext[:, :, 64:66], bvb_sb[:, :, 64:66])

                        kpps = ps_kproj.tile([128, G], f32)
                        for h in range(HPC):
                            nc.tensor.matmul(
                                kpps[:, h * 64 : (h + 1) * 64],
                                (k_sb[:, h, nsl]), (extp_sb[:]),
                                start=True, stop=True,
                            )
                        kp = kp_pool.tile([128, G], f32r)
                        nc.scalar.activation(kp[:], kpps[:], AF.Exp, bias=ebias_sb[:])

                        first = c == 0 and nt == 0
                        last = c == nchunks - 1 and nt == ntiles - 1
                        for h in range(HPC):
                            # one accumulation group per psum bank: start only
                            # with the bank's first matmul, stop with its last
                            nc.tensor.matmul(
                                kv_ps[h // 4][:, h % 4, :],
                                (kp[:, h * 64 : (h + 1) * 64]),
                                (vext[:, h, :]),
                                start=first and h % 4 == 0,
                                stop=last and h % 4 == 3,
                            )

                nc.vector.tensor_copy(kvT_sb[:, 0:4, :], kv_ps_a[:])
                nc.vector.tensor_copy(kvT_sb[:, 4:8, :], kv_ps_b[:])

            # ---------------- PASS 2: q' -> out -> projection ----------------
            with (
                tc.tile_pool(name="xt2", bufs=2) as xt_pool,
                tc.tile_pool(name="qsb", bufs=2) as q_pool,
                tc.tile_pool(name="qprime", bufs=2) as qp_pool,
                tc.tile_pool(name="normrow", bufs=2) as nr_pool,
                tc.tile_pool(name="recipb", bufs=2) as rb_pool,
                tc.tile_pool(name="ohT", bufs=2) as oh_pool,
                tc.tile_pool(name="ps_big2", bufs=3, space="PSUM") as ps_big,
                tc.tile_pool(name="ps_s64", bufs=3, space="PSUM") as ps_s64,
            ):
                for c in range(nchunks):
                    xt = xt_pool.tile([128, 8, CHUNK], f32r)
                    nc.sync.dma_start(
                        xt[:],
                        xT.ap().rearrange("(dt p) n -> p dt n", p=128)[
                            :, :, c * CHUNK : (c + 1) * CHUNK
                        ],
                    )
                    q_sb = q_pool.tile([128, 8, CHUNK], f32r)
                    for jt in range(4):
                        ps = ps_big.tile([128, CHUNK], f32, tag="big")
                        for dt in range(8):
                            nc.tensor.matmul(
                                ps[:],
                                (w_sb[:, dt, jt * 128 : (jt + 1) * 128]),
                                (xt[:, dt, :]),
                                start=(dt == 0),
                                stop=(dt == 7),
                            )
                        for hh in range(2):
                            h = 2 * jt + hh
                            nc.scalar.activation(
                                q_sb[0:64, h, :], ps[hh * 64 : hh * 64 + 64, :],
                                AF.Identity, bias=bqk_sb[:, h : h + 1],
                            )
                            nc.scalar.activation(
                                q_sb[64:128, h, :], q_sb[0:64, h, :], AF.Square,
                            )

                    qprime = qp_pool.tile([64, HPC, CHUNK], f32r)
                    for h in range(HPC):
                        qps = ps_s64.tile([64, CHUNK], f32, tag="s64")
                        nc.tensor.matmul(
                            qps[:], (extp_sb[:]), (q_sb[:, h, :]),
                            start=True, stop=True,
                        )
                        nc.scalar.activation(qprime[:, h, :], qps[:], AF.Exp, bias=ebias_sb[0:64, :])

                    # out^T rows 0:64 = unnormalized head output, row 64 =
                    # the normalizer (from the ksum column folded into kv)
                    ohT = oh_pool.tile([128, 4, CHUNK], f32r)
                    for h in range(HPC):
                        ot = ps_s64.tile([65, CHUNK], f32, tag="s64")
                        nc.tensor.matmul(
                            ot[:], (kvT_sb[:, h, 0:65]),
                            (qprime[:, h, :]),
                        )
                        nsc = nr_pool.tile([1, CHUNK], f32, tag="nsc")
                        nc.vector.tensor_scalar_add(nsc[:], ot[64:65, :], 1e-6)
                        rsc = nr_pool.tile([1, CHUNK], f32, tag="rsc")
                        nc.vector.reciprocal_approx_fast(rsc[:], nsc[:])
                        rb = rb_pool.tile([64, CHUNK], f32)
                        nc.gpsimd.partition_broadcast(rb[:], rsc[:])
                        nc.vector.tensor_mul(
                            ohT[(h % 2) * 64 : (h % 2) * 64 + 64, h // 2, :],
                            ot[0:64, :], rb[:],
                        )

                    for otile in range(8):
                        fps = ps_big.tile([128, CHUNK], f32, tag="big")
                        for ht in range(4):
                            nc.tensor.matmul(
                                fps[:],
                                (wp_sb[:, ht, otile * 128 : (otile + 1) * 128]),
                                (ohT[:, ht, :]),
                                start=(ht == 0),
                                stop=(ht == 3),
                            )
                        fout = oh_pool.tile([128, CHUNK], f32, tag="fout")
                        nc.vector.tensor_copy(fout[:], fps[:])
                        nc.sync.dma_start(
                            pout[otile * 128 : (otile + 1) * 128,
                                 c * CHUNK : (c + 1) * CHUNK],
                            fout[:],
                        )

    nc.compile()
    return nc


_NC_CACHE = {}


def _get_nc(n_tokens=N):
    if n_tokens not in _NC_CACHE:
        _NC_CACHE[n_tokens] = _build(n_tokens)
    return _NC_CACHE[n_tokens]


def _make_in_maps(x, w_qkv, b_qkv, w_proj, proj_mat):
    """Host-side sharding: one input map per core."""
    in_maps = []
    for c in range(8):
        b, g = c // 2, c % 2
        gs = slice(g * G, (g + 1) * G)
        wq = w_qkv[:, 0:D][:, gs]
        wk = w_qkv[:, D : 2 * D][:, gs]
        wv = w_qkv[:, 2 * D : 3 * D][:, gs]
        bq = b_qkv[0:D][gs]
        bk = b_qkv[D : 2 * D][gs]
        bv = b_qkv[2 * D : 3 * D][gs]
        bvb = np.zeros((128, HPC, 66), np.float32)
        for h in range(HPC):
            bvb[:, h, 0:64] = bv[h * 64 : (h + 1) * 64][None, :]
            bvb[:, h, 64] = 1.0
        bqk = np.concatenate(
            [bq.reshape(HPC, HD).T, bk.reshape(HPC, HD).T], axis=1
        )  # [64, 16]: cols 0..7 q-head biases, 8..15 k-head biases
        in_maps.append(
            {
                "xT": np.ascontiguousarray(x[b].T),
                "w_qkv": np.ascontiguousarray(np.concatenate([wq, wk, wv], axis=1)),
                "b_qk": np.ascontiguousarray(bqk.astype(np.float32)),
                "w_proj": np.ascontiguousarray(w_proj[gs, :]),
                "extp": np.concatenate(
                    [proj_mat, np.full((HD, M), -0.5, np.float32)], axis=0
                ),
                "bvb": bvb.reshape(128, HPC * 66),
            }
        )
    return in_maps


def kernel(x, w_qkv, b_qkv, w_proj, b_proj, proj_mat, trace=False):
    from concourse.bass_utils import run_bass_kernel_spmd

    x = np.asarray(x, np.float32)
    w_qkv = np.asarray(w_qkv, np.float32)
    b_qkv = np.asarray(b_qkv, np.float32)
    w_proj = np.asarray(w_proj, np.float32)
    b_proj = np.asarray(b_proj, np.float32)
    proj_mat = np.asarray(proj_mat, np.float32)

    nc = _get_nc(N)
    in_maps = _make_in_maps(x, w_qkv, b_qkv, w_proj, proj_mat)
    res = run_bass_kernel_spmd(nc, in_maps, core_ids=list(range(8)), trace=trace)
    out = np.empty((B, N, D), np.float32)
    for b in range(B):
        pT = res.results[2 * b]["pout"] + res.results[2 * b + 1]["pout"]
        out[b] = pT.T + b_proj[None, :]
    kernel.last_result = res
    return out


# revision 15
# speedup vs baseline: 1.2651x; 1.1097x over previous
"""FAVOR+ attention kernel for 8 Trainium2 NeuronCores.

Sharding: core c handles batch b = c//2 and head-group g = c%2 (8 of 16
heads). Each core computes its group's partial output projection
out_part^T [D, N]; the host sums the two partials per batch and
transposes back.

Layout convention on-chip: "feature-major" (transposed) tensors — the
qkv projections produce qT/kT [j, n] with j on partitions, which feeds
the Performer feature map and the final projection without any on-chip
transposes (x arrives pre-transposed from the host).
"""

import math

import numpy as np

B, N, D = 4, 4096, 1024
H = 16
HD = 64
M = HD
HPC = 8          # heads per core
G = HPC * HD     # 512 cols per group per q/k/v
STAB = 1e-6
RATIO = 1.0 / math.sqrt(M)
EXP_BIAS = STAB + math.log(RATIO)   # exp(x + STAB) * ratio == exp(x + STAB + ln ratio)

CHUNK = 512
NT = 128




def _build(n_tokens=N):
    import concourse.bass as bass
    import concourse.mybir as mybir
    import concourse.tile as tile
    from concourse import bacc

    f32 = mybir.dt.float32
    f32r = mybir.dt.float32r
    AF = mybir.ActivationFunctionType

    nchunks = n_tokens // CHUNK
    ntiles = CHUNK // NT  # 4

    nc = bacc.Bacc("TRN2", target_bir_lowering=False, debug=False, num_devices=8)

    xT = nc.dram_tensor("xT", [D, n_tokens], f32r, kind="ExternalInput")
    w_qkv = nc.dram_tensor("w_qkv", [D, 3 * G], f32r, kind="ExternalInput")
    b_qk = nc.dram_tensor("b_qk", [HD, 2 * HPC], f32, kind="ExternalInput")
    w_proj = nc.dram_tensor("w_proj", [G, D], f32r, kind="ExternalInput")
    extp = nc.dram_tensor("extp", [128, M], f32r, kind="ExternalInput")
    bvb = nc.dram_tensor("bvb", [128, HPC * 66], f32, kind="ExternalInput")
    pout = nc.dram_tensor("pout", [D, n_tokens], f32, kind="ExternalOutput")

    with tile.TileContext(nc) as tc:
        with (
            tc.tile_pool(name="const", bufs=1) as cpool,
            tc.tile_pool(name="kv_sb", bufs=1) as kvsb_pool,
        ):
            # weights resident in SBUF
            w_sb = cpool.tile([128, 8, 3 * G], f32r)
            nc.sync.dma_start(w_sb[:], w_qkv.ap().rearrange("(dt p) j -> p dt j", p=128))
            wp_sb = cpool.tile([128, 4, D], f32r)
            nc.sync.dma_start(wp_sb[:], w_proj.ap().rearrange("(ht p) o -> p ht o", p=128))
            # NOTE every fp32r matmul operand must sit at partition base 0:
            # mixing lhsT base partitions across fp32r matmuls hangs the
            # device (probed on HW). extP = [P ; -0.5] so that
            # extP.T @ [k ; k^2] = P.T k - 0.5*sum(k^2) in one matmul with
            # lhsT/rhs both at base 0.
            extp_sb = cpool.tile([128, M], f32r)
            nc.sync.dma_start(extp_sb[:], extp[:])
            bqk_sb = cpool.tile([HD, 2 * HPC], f32)
            nc.sync.dma_start(bqk_sb[:], b_qk[:])
            bvb_sb = cpool.tile([128, HPC, 66], f32)
            nc.sync.dma_start(bvb_sb[:], bvb.ap().rearrange("p (h e) -> p h e", h=HPC))
            ebias_sb = cpool.tile([128, 1], f32)
            nc.vector.memset(ebias_sb[:], EXP_BIAS)

            kvT_sb = kvsb_pool.tile([HD, HPC, 66], f32r)

            # ---------------- PASS 1: k', v -> kv state ----------------
            with (
                tc.tile_pool(name="xt1", bufs=2) as xt_pool,
                tc.tile_pool(name="ksb", bufs=2) as k_pool,
                tc.tile_pool(name="vext", bufs=2) as v_pool,
                tc.tile_pool(name="kprime", bufs=2) as kp_pool,
                tc.tile_pool(name="ps_kq1", bufs=2, space="PSUM") as ps_kq,
                tc.tile_pool(name="ps_v", bufs=2, space="PSUM") as ps_v,
                tc.tile_pool(name="ps_kproj", bufs=2, space="PSUM") as ps_kproj,
                tc.tile_pool(name="ps_kv", bufs=1, space="PSUM") as ps_kv,
            ):
                kv_ps_a = ps_kv.tile([HD, 4, 66], f32, tag="kva")
                kv_ps_b = ps_kv.tile([HD, 4, 66], f32, tag="kvb")
                kv_ps = (kv_ps_a, kv_ps_b)

                for c in range(nchunks):
                    xt = xt_pool.tile([128, 8, CHUNK], f32r)
                    nc.sync.dma_start(
                        xt[:],
                        xT.ap().rearrange("(dt p) n -> p dt n", p=128)[
                            :, :, c * CHUNK : (c + 1) * CHUNK
                        ],
                    )
                    k_sb = k_pool.tile([128, 8, CHUNK], f32r)
                    for jt in range(4):
                        ps = ps_kq.tile([128, CHUNK], f32)
                        for dt in range(8):
                            nc.tensor.matmul(
                                ps[:],
                                (w_sb[:, dt, G + jt * 128 : G + (jt + 1) * 128]),
                                (xt[:, dt, :]),
                                start=(dt == 0),
                                stop=(dt == 7),
                            )
                        for hh in range(2):
                            h = 2 * jt + hh
                            nc.scalar.activation(
                                k_sb[0:64, h, :], ps[hh * 64 : hh * 64 + 64, :],
                                AF.Identity, bias=bqk_sb[:, HPC + h : HPC + h + 1],
                            )
                            nc.scalar.activation(
                                k_sb[64:128, h, :], ps[hh * 64 : hh * 64 + 64, :],
                                AF.Square, bias=bqk_sb[:, HPC + h : HPC + h + 1],
                            )

                    for nt in range(ntiles):
                        nsl = slice(nt * NT, (nt + 1) * NT)
                        vps = ps_v.tile([128, G], f32)
                        for dt in range(8):
                            nc.tensor.matmul(
                                vps[:],
                                (xt[:, dt, nsl]),
                                (w_sb[:, dt, 2 * G : 3 * G]),
                                start=(dt == 0),
                                stop=(dt == 7),
                            )
                        vext = v_pool.tile([128, HPC, 66], f32r)
                        nc.vector.tensor_add(
                            vext[:, :, 0:64],
                            vps.rearrange("p (h d) -> p h d", h=HPC),
                            bvb_sb[:, :, 0:64],
                        )
                        nc.vector.tensor_copy(vext[:, :, 64:66], bvb_sb[:, :, 64:66])

                        kpps = ps_kproj.tile([128, G], f32)
                        for h in range(HPC):
                            nc.tensor.matmul(
                                kpps[:, h * 64 : (h + 1) * 64],
                                (k_sb[:, h, nsl]), (extp_sb[:]),
                                start=True, stop=True,
                            )
                        kp = kp_pool.tile([128, G], f32r)
                        nc.scalar.activation(kp[:], kpps[:], AF.Exp, bias=ebias_sb[:])

                        first = c == 0 and nt == 0
                        last = c == nchunks - 1 and nt == ntiles - 1
                        for h in range(HPC):
                            # one accumulation group per psum bank: start only
                            # with the bank's first matmul, stop with its last
                            nc.tensor.matmul(
                                kv_ps[h // 4][:, h % 4, :],
                                (kp[:, h * 64 : (h + 1) * 64]),
                                (vext[:, h, :]),
                                start=first and h % 4 == 0,
                                stop=last and h % 4 == 3,
                            )

                nc.vector.tensor_copy(kvT_sb[:, 0:4, :], kv_ps_a[:])
                nc.vector.tensor_copy(kvT_sb[:, 4:8, :], kv_ps_b[:])

            # ---------------- PASS 2: q' -> out -> projection ----------------
            with (
                tc.tile_pool(name="xt2", bufs=2) as xt_pool,
                tc.tile_pool(name="qsb", bufs=2) as q_pool,
                tc.tile_pool(name="qprime", bufs=2) as qp_pool,
                tc.tile_pool(name="normrow", bufs=2) as nr_pool,
                tc.tile_pool(name="recipb", bufs=2) as rb_pool,
                tc.tile_pool(name="ohT", bufs=2) as oh_pool,
                tc.tile_pool(name="ps_big2", bufs=2, space="PSUM") as ps_big,
                tc.tile_pool(name="ps_qproj", bufs=2, space="PSUM") as ps_qproj,
                tc.tile_pool(name="ps_ot", bufs=4, space="PSUM") as ps_ot,
            ):
                for c in range(nchunks):
                    xt = xt_pool.tile([128, 8, CHUNK], f32r)
                    nc.sync.dma_start(
                        xt[:],
                        xT.ap().rearrange("(dt p) n -> p dt n", p=128)[
                            :, :, c * CHUNK : (c + 1) * CHUNK
                        ],
                    )
                    q_sb = q_pool.tile([128, 8, CHUNK], f32r)
                    for jt in range(4):
                        ps = ps_big.tile([128, CHUNK], f32, tag="big")
                        for dt in range(8):
                            nc.tensor.matmul(
                                ps[:],
                                (w_sb[:, dt, jt * 128 : (jt + 1) * 128]),
                                (xt[:, dt, :]),
                                start=(dt == 0),
                                stop=(dt == 7),
                            )
                        for hh in range(2):
                            h = 2 * jt + hh
                            nc.scalar.activation(
                                q_sb[0:64, h, :], ps[hh * 64 : hh * 64 + 64, :],
                                AF.Identity, bias=bqk_sb[:, h : h + 1],
                            )
                            nc.scalar.activation(
                                q_sb[64:128, h, :], ps[hh * 64 : hh * 64 + 64, :],
                                AF.Square, bias=bqk_sb[:, h : h + 1],
                            )

                    qprime = qp_pool.tile([64, HPC, CHUNK], f32r)
                    for h in range(HPC):
                        qps = ps_qproj.tile([64, CHUNK], f32, tag="qproj")
                        nc.tensor.matmul(
                            qps[:], (extp_sb[:]), (q_sb[:, h, :]),
                            start=True, stop=True,
                        )
                        nc.scalar.activation(qprime[:, h, :], qps[:], AF.Exp, bias=ebias_sb[0:64, :])

                    # out^T rows 0:64 = unnormalized head output, row 64 =
                    # the normalizer (from the ksum column folded into kv)
                    ohT = oh_pool.tile([128, 4, CHUNK], f32r)
                    for h in range(HPC):
                        ot = ps_ot.tile([65, CHUNK], f32, tag="ot")
                        nc.tensor.matmul(
                            ot[:], (kvT_sb[:, h, 0:65]),
                            (qprime[:, h, :]),
                        )
                        nsc = nr_pool.tile([1, CHUNK], f32, tag="nsc")
                        nc.vector.tensor_scalar_add(nsc[:], ot[64:65, :], 1e-6)
                        rsc = nr_pool.tile([1, CHUNK], f32, tag="rsc")
                        nc.vector.reciprocal_approx_fast(rsc[:], nsc[:])
                        rb = rb_pool.tile([64, CHUNK], f32)
                        nc.gpsimd.partition_broadcast(rb[:], rsc[:])
                        nc.vector.tensor_mul(
                            ohT[(h % 2) * 64 : (h % 2) * 64 + 64, h // 2, :],
                            ot[0:64, :], rb[:],
                        )

                    for otile in range(8):
                        fps = ps_big.tile([128, CHUNK], f32, tag="big")
                        for ht in range(4):
                            nc.tensor.matmul(
                                fps[:],
                                (wp_sb[:, ht, otile * 128 : (otile + 1) * 128]),
                                (ohT[:, ht, :]),
                                start=(ht == 0),
                                stop=(ht == 3),
                            )
                        fout = oh_pool.tile([128, CHUNK], f32, tag="fout")
                        nc.vector.tensor_copy(fout[:], fps[:])
                        nc.sync.dma_start(
                            pout[otile * 128 : (otile + 1) * 128,
                                 c * CHUNK : (c + 1) * CHUNK],
                            fout[:],
                        )

    nc.compile()
    return nc


_NC_CACHE = {}


def _get_nc(n_tokens=N):
    if n_tokens not in _NC_CACHE:
        _NC_CACHE[n_tokens] = _build(n_tokens)
    return _NC_CACHE[n_tokens]


def _make_in_maps(x, w_qkv, b_qkv, w_proj, proj_mat):
    """Host-side sharding: one input map per core."""
    in_maps = []
    for c in range(8):
        b, g = c // 2, c % 2
        gs = slice(g * G, (g + 1) * G)
        wq = w_qkv[:, 0:D][:, gs]
        wk = w_qkv[:, D : 2 * D][:, gs]
        wv = w_qkv[:, 2 * D : 3 * D][:, gs]
        bq = b_qkv[0:D][gs]
        bk = b_qkv[D : 2 * D][gs]
        bv = b_qkv[2 * D : 3 * D][gs]
        bvb = np.zeros((128, HPC, 66), np.float32)
        for h in range(HPC):
            bvb[:, h, 0:64] = bv[h * 64 : (h + 1) * 64][None, :]
            bvb[:, h, 64] = 1.0
        bqk = np.concatenate(
            [bq.reshape(HPC, HD).T, bk.reshape(HPC, HD).T], axis=1
        )  # [64, 16]: cols 0..7 q-head biases, 8..15 k-head biases
        in_maps.append(
            {
                "xT": np.ascontiguousarray(x[b].T),
                "w_qkv": np.ascontiguousarray(np.concatenate([wq, wk, wv], axis=1)),
                "b_qk": np.ascontiguousarray(bqk.astype(np.float32)),
                "w_proj": np.ascontiguousarray(w_proj[gs, :]),
                "extp": np.concatenate(
                    [proj_mat, np.full((HD, M), -0.5, np.float32)], axis=0
                ),
                "bvb": bvb.reshape(128, HPC * 66),
            }
        )
    return in_maps


def kernel(x, w_qkv, b_qkv, w_proj, b_proj, proj_mat, trace=False):
    from concourse.bass_utils import run_bass_kernel_spmd

    x = np.asarray(x, np.float32)
    w_qkv = np.asarray(w_qkv, np.float32)
    b_qkv = np.asarray(b_qkv, np.float32)
    w_proj = np.asarray(w_proj, np.float32)
    b_proj = np.asarray(b_proj, np.float32)
    proj_mat = np.asarray(proj_mat, np.float32)

    nc = _get_nc(N)
    in_maps = _make_in_maps(x, w_qkv, b_qkv, w_proj, proj_mat)
    res = run_bass_kernel_spmd(nc, in_maps, core_ids=list(range(8)), trace=trace)
    out = np.empty((B, N, D), np.float32)
    for b in range(B):
        pT = res.results[2 * b]["pout"] + res.results[2 * b + 1]["pout"]
        out[b] = pT.T + b_proj[None, :]
    kernel.last_result = res
    return out
